# revision 11
# baseline (speedup 1.0000x reference)
"""Causal self-attention (B=2, T=2048, C=1024, H=16) on 8 trn2 NeuronCores.

Sharding: Megatron tensor-parallel x data-parallel. Core cid = 4*b + g
handles batch b (of 2) and head group g (4 heads of 16): its heads' QKV
projection, attention, and the partial output projection (w_proj rows for
those heads). Host sums the 4 partials per batch and adds the bias.

Layout: everything stays transposed (host passes x[b].T) so no on-device
transposes are needed:
  - q^T/k^T come from  lhsT=w_qk[c,j],  rhs=xT[c,t]   (head pair per 128)
  - V      comes from  lhsT=xT[c,t],    rhs=w_v[c,j]  (+ persistent ones col)
  - S^T    comes from  lhsT=k^T[d,tk],  rhs=q^T[d,tq] at K=64. The two heads
           of a pair live at rows 0:64 / 64:128 of one 128-partition tile, so
           their S matmuls auto-derive tile_position (0,0)/(64,0) — emitted
           back-to-back they run CONCURRENTLY on PE row-tiles T0/T8 (2x),
           writing the two banks of one [P,2,TCH] PSUM tile.
  - y^T    comes from  lhsT=V[tk,d|1],  rhs=P^T[tk,tq] (ones col -> l)
  - out    comes from  lhsT=y^T[d,t],   rhs=w_proj[d,c]

Scale/bias folding (host): wq pre-scaled by 1/sqrt(hd); k-bias is
softmax-invariant when q-bias is 0 (dropped); v-bias folds exactly into
the host-side output bias (softmax weights sum to 1). Nonzero q-bias is
rejected.

Softmax skips max-subtraction (logits ~N(0,1), exp safe). exp runs on ACT
reading both PSUM banks of a pair-unit in one instruction (1024 cols).
Causal masking multiplies the diagonal staircase blocks (bf16, all-SBUF)
after exp; fully-masked column prefixes are skipped in S/exp/AV. The
denominator l is harvested from the V ones column; the epilogue stages
py->SBUF in one fp32 copy per head (frees the PSUM bank fast), then
reciprocal_approx_fast -> bf16 cast (DVE) -> 64-channel partition
broadcast (GpSimd) -> fused normalize-multiply into yT (DVE). No slow
GpSimd casts on the critical path.

Scheduling: one continuous PE stream. Pair-units are software-pipelined
with lookahead 2 (S pair -> exp pair -> [mask] -> AV pair two units
later); QKV of chunk a+1 and projection of chunk a-1 are paced into the
stream as filler. ~8 junk matmuls on a zeroed tile warm the PE HAM clock
gate during the initial DMA wait. Startup DMAs are spread over 4 queues
(sync/scalar/vector/gpsimd) in kt-order so the first projection matmuls
start as early as possible; x-chunk prefetch DMAs are issued one chunk
ahead on the gpsimd queue. Projection PSUM evacuations run on DVE during
steady state (ACT is exp-bound) and alternate ACT/DVE in the tail.
"""

import numpy as np

B, T, C, H = 2, 2048, 1024, 16
HD = C // H  # 64
P = 128
NKT = C // P  # 8 k-tiles over the embedding dim
TCH = 512  # t-chunk (q) width
NCH = T // TCH  # 4 q-chunks
NTB = T // P  # 16 t-blocks (k) per sequence
HPC = 4  # heads per core
DC = HPC * HD  # 256 head dims per core
NWARM = 5  # HAM warmup junk matmuls at startup

_CACHE = {}


def _build():
    import concourse.mybir as mybir
    from concourse import bacc
    from concourse.tile import TileContext

    F32 = mybir.dt.float32
    BF16 = mybir.dt.bfloat16
    AF = mybir.ActivationFunctionType

    nc = bacc.Bacc("TRN2", target_bir_lowering=False, debug=False)

    xT = nc.dram_tensor("xT", (C, T), BF16, kind="ExternalInput")
    wqk = nc.dram_tensor("wqk", (C, 4 * P), BF16, kind="ExternalInput")
    wv = nc.dram_tensor("wv", (C, DC), BF16, kind="ExternalInput")
    wproj = nc.dram_tensor("wproj", (DC, C), BF16, kind="ExternalInput")
    masks = nc.dram_tensor("masks", (P, 4 * TCH), BF16, kind="ExternalInput")
    out = nc.dram_tensor("out", (T, C), F32, kind="ExternalOutput")

    xT_r = xT[:].rearrange("(kt p) t -> p kt t", p=P)
    wqk_r = wqk[:].rearrange("(kt p) j -> p kt j", p=P)
    wv_r = wv[:].rearrange("(kt p) j -> p kt j", p=P)
    wproj_r = wproj[:].rearrange("(kt p) n -> p kt n", p=P)

    with TileContext(nc) as tc:
        with (
            tc.tile_pool(name="persist", bufs=1) as pp,
        ):
            wqk_sb = pp.tile([P, NKT, 4 * P], BF16)
            wv_sb = pp.tile([P, NKT, DC], BF16)
            wproj_sb = pp.tile([P, DC // P, C], BF16)
            masks_sb = pp.tile([P, 4 * TCH], BF16)
            # q^T/k^T head-pair tiles: pair hp rows 0:64=head 2hp, 64:128=2hp+1
            qt2 = [pp.tile([P, T], BF16, name=f"qt{i}") for i in range(2)]
            kt2 = [pp.tile([P, T], BF16, name=f"kt{i}") for i in range(2)]
            v_sb = pp.tile([P, NTB, HPC, HD + 1], BF16)  # ones col at HD
            yT_sb = pp.tile([P, DC // P, T], BF16)
            wzero = pp.tile([P, P + TCH], BF16)  # HAM warmup junk operands

            nc.vector.memset(wzero[:], 0.0)
            nc.vector.memset(v_sb[:, :, :, HD : HD + 1], 1.0)

            with (
                tc.tile_pool(name="xin", bufs=2) as xpool,
                tc.tile_pool(name="ps_s", bufs=2, space="PSUM") as ps_pool,
                tc.tile_pool(name="ps_y", bufs=2, space="PSUM") as py_pool,
                tc.tile_pool(name="ps_o", bufs=2, space="PSUM") as po_pool,
                tc.tile_pool(name="pt", bufs=4) as pt_pool,
                tc.tile_pool(name="outs", bufs=3) as o_pool,
                tc.tile_pool(name="stg", bufs=2) as stg_pool,
                tc.tile_pool(name="rec", bufs=4) as rec_pool,
                tc.tile_pool(name="rbf", bufs=2) as rbf_pool,
                tc.tile_pool(name="rb", bufs=2) as rb_pool,
            ):
                # ---------- QKV building blocks ----------
                def qk1(x_t, a2, jt):
                    """Project q (jt 0,1) or k (jt 2,3) for head pair jt%2."""
                    ch2 = slice(a2 * TCH, (a2 + 1) * TCH)
                    pq = po_pool.tile([P, TCH], F32, tag="po", name=f"pq{a2}_{jt}")
                    for kt in range(NKT):
                        nc.tensor.matmul(
                            pq[:],
                            wqk_sb[:, kt, jt * P : (jt + 1) * P],
                            x_t[:, kt, :],
                            start=(kt == 0),
                            stop=(kt == NKT - 1),
                        )
                    dst = qt2[jt] if jt < 2 else kt2[jt - 2]
                    nc.scalar.activation(dst[:, ch2], pq[:], AF.Copy)

                def v1(x_t, a2, tb):
                    pv = po_pool.tile(
                        [P, HPC, HD], F32, tag="po", name=f"pv{a2}_{tb}"
                    )
                    tg = a2 * (TCH // P) + tb
                    for kt in range(NKT):
                        nc.tensor.matmul(
                            pv[:],
                            x_t[:, kt, tb * P : (tb + 1) * P],
                            wv_sb[:, kt, :],
                            start=(kt == 0),
                            stop=(kt == NKT - 1),
                        )
                    nc.vector.tensor_copy(v_sb[:, tg, :, 0:HD], pv[:])

                def make_qkv(a2):
                    """Emit x DMA now (gpsimd queue); return qkv step list."""
                    ch2 = slice(a2 * TCH, (a2 + 1) * TCH)
                    x_t = xpool.tile(
                        [P, NKT, TCH], BF16, tag="x_tile", name=f"x{a2}"
                    )
                    nc.gpsimd.dma_start(x_t[:, 0:4, :], xT_r[:, 0:4, ch2])
                    nc.gpsimd.dma_start(x_t[:, 4:NKT, :], xT_r[:, 4:NKT, ch2])
                    qk = [
                        (lambda jt=jt: qk1(x_t, a2, jt)) for jt in (0, 2, 1, 3)
                    ]
                    v = [(lambda tb=tb: v1(x_t, a2, tb)) for tb in range(4)]
                    return qk, v

                # ---------- projection steps ----------
                def proj_steps(a, tail=False):
                    steps = []

                    def pstep(tb, ncx, cp_eng):
                        tg = a * (TCH // P) + tb
                        po = po_pool.tile(
                            [P, TCH], F32, tag="po", name=f"po{a}_{tb}_{ncx}"
                        )
                        for kt in range(DC // P):
                            nc.tensor.matmul(
                                po[:],
                                yT_sb[:, kt, tg * P : (tg + 1) * P],
                                wproj_sb[:, kt, ncx * TCH : (ncx + 1) * TCH],
                                start=(kt == 0),
                                stop=(kt == DC // P - 1),
                            )
                        ot = o_pool.tile([P, TCH], F32, tag="osb")
                        if cp_eng == "act":
                            nc.scalar.activation(ot[:], po[:], AF.Copy)
                        else:
                            nc.vector.tensor_copy(ot[:], po[:])
                        nc.sync.dma_start(
                            out[tg * P : (tg + 1) * P, ncx * TCH : (ncx + 1) * TCH],
                            ot[:],
                        )

                    for i, (tb, ncx) in enumerate(
                        (tb, ncx) for tb in range(TCH // P) for ncx in range(2)
                    ):
                        eng = ("act" if i % 2 == 0 else "dve") if tail else "dve"
                        steps.append(
                            lambda tb=tb, ncx=ncx, eng=eng: pstep(tb, ncx, eng)
                        )
                    return steps

                # ---------- attention chunk ----------
                def emit_chunk(a, carry):
                    ch = slice(a * TCH, (a + 1) * TCH)
                    nblk = 4 * a + 4
                    LOOK = 2

                    state = {}  # (hp,j) -> (pt, c0)
                    pys = {}  # (hp,s) -> py

                    def emit_S(hp, j):
                        r = j - 4 * a
                        c0 = P * r if r >= 0 else 0
                        ps = ps_pool.tile([P, 2, TCH], F32, tag="ps")
                        pt = pt_pool.tile([P, 2, TCH], BF16, tag="pt")
                        # the two 64-row matmuls run concurrently on PE row
                        # tiles (0,0)/(64,0), one PSUM bank each
                        for s in range(2):
                            hr = 64 * s
                            nc.tensor.matmul(
                                ps[:, s, c0:],
                                kt2[hp][hr : hr + 64, j * P : (j + 1) * P],
                                qt2[hp][hr : hr + 64, a * TCH + c0 : (a + 1) * TCH],
                                start=True,
                                stop=True,
                            )
                        if r >= 0:
                            nc.scalar.activation(
                                pt[:, :, c0:], ps[:, :, c0:], AF.Exp
                            )
                            for s in range(2):
                                nc.vector.tensor_mul(
                                    pt[:, s, c0:],
                                    pt[:, s, c0:],
                                    masks_sb[:, r * TCH + c0 : (r + 1) * TCH],
                                )
                        else:
                            nc.scalar.activation(pt[:], ps[:], AF.Exp)
                        state[(hp, j)] = (pt, c0)

                    def emit_AV(hp, j):
                        pt, c0 = state.pop((hp, j))
                        for s in range(2):
                            if j == 0:
                                pys[(hp, s)] = py_pool.tile(
                                    [P, TCH], F32, tag="py", name=f"py{a}_{hp}_{s}"
                                )
                            nc.tensor.matmul(
                                pys[(hp, s)][0 : HD + 1, c0:],
                                v_sb[:, j, 2 * hp + s, :],
                                pt[:, s, c0:],
                                start=(j == 0),
                                stop=(j == nblk - 1),
                            )

                    def epilogue(hp):
                        # stage py -> SBUF (bf16 y rows + fp32 l row frees the
                        # PSUM bank fast), then recip/cast/bcast/normalize.
                        # The very last epilogue has nothing left to overlap
                        # with, so offload part of its chain to the idle ACT.
                        last = a == NCH - 1 and hp == 1
                        stgs = []
                        for s in range(2):
                            py = pys.pop((hp, s))
                            stg = stg_pool.tile([HD, TCH], BF16, tag="stg")
                            if last and s == 1:
                                nc.scalar.activation(stg[:], py[0:HD, :], AF.Copy)
                            else:
                                nc.vector.tensor_copy(stg[:], py[0:HD, :])
                            lrow = rec_pool.tile([1, TCH], F32, tag="lrow")
                            nc.vector.tensor_copy(lrow[:], py[HD : HD + 1, :])
                            stgs.append((stg, lrow))
                        for s in range(2):
                            stg, lrow = stgs[s]
                            hr = 64 * s
                            rec = rec_pool.tile([1, TCH], F32, tag="rec")
                            with nc.allow_low_precision(
                                reason="approx recip of softmax denominator"
                            ):
                                nc.vector.reciprocal_approx_fast(
                                    rec[:], lrow[:]
                                )
                            rbf = rbf_pool.tile([1, TCH], BF16, tag="rbf")
                            if last:
                                nc.scalar.activation(rbf[:], rec[:], AF.Copy)
                            else:
                                nc.vector.tensor_copy(rbf[:], rec[:])
                            rb = rb_pool.tile([64, TCH], BF16, tag="rb")
                            nc.gpsimd.partition_broadcast(rb[:], rbf[:])
                            nc.vector.tensor_mul(
                                yT_sb[hr : hr + 64, hp, ch], stg[:], rb[:]
                            )

                    units = [(hp, j) for hp in range(2) for j in range(nblk)]
                    slots = len(units) + LOOK
                    nf = len(carry)
                    popped = 0
                    for i in range(slots):
                        if i < len(units):
                            emit_S(*units[i])
                        if i >= LOOK:
                            hp, j = units[i - LOOK]
                            emit_AV(hp, j)
                            if j == nblk - 1:
                                epilogue(hp)
                        tgt = min(nf, (i + 1) * nf // slots + 1)
                        while popped < tgt:
                            carry[popped]()
                            popped += 1
                    while popped < nf:
                        carry[popped]()
                        popped += 1

                # ---------- prologue: startup DMAs on 3 queues ----------
                # fine-grained per-2-kt transfers: the Tile dependency is
                # per-DMA, so small pieces let the first matmuls start as
                # soon as their slice lands instead of waiting for 512KB
                x0 = xpool.tile([P, NKT, TCH], BF16, tag="x_tile", name="x0")
                for kt in range(0, NKT, 2):
                    nc.sync.dma_start(
                        x0[:, kt : kt + 2, :], xT_r[:, kt : kt + 2, 0:TCH]
                    )
                    nc.scalar.dma_start(
                        wqk_sb[:, kt : kt + 2, :], wqk_r[:, kt : kt + 2, :]
                    )
                nc.gpsimd.dma_start(wv_sb[:, 0:4, :], wv_r[:, 0:4, :])
                nc.gpsimd.dma_start(wv_sb[:, 4:NKT, :], wv_r[:, 4:NKT, :])
                nc.gpsimd.dma_start(masks_sb[:], masks[:])

                # HAM warmup: junk matmuls on the zeroed tile keep the PE
                # busy (and the clock gate warm) while DMAs or the final
                # epilogue are in flight
                def junk_mms(n, label):
                    for w in range(n):
                        pw = po_pool.tile(
                            [P, TCH], F32, tag="po", name=f"{label}{w}"
                        )
                        nc.tensor.matmul(
                            pw[:],
                            wzero[:, 0:P],
                            wzero[:, P : P + TCH],
                            start=True,
                            stop=True,
                        )

                junk_mms(NWARM, "warm")

                # QKV(0) inline: heads 0,1 q/k first, then V, then heads 2,3
                qk0, v0 = [
                    (lambda jt=jt: qk1(x0, 0, jt)) for jt in (0, 2, 1, 3)
                ], [(lambda tb=tb: v1(x0, 0, tb)) for tb in range(4)]
                qk0[0]()
                qk0[1]()
                for st in v0:
                    st()
                qk0[2]()
                qk0[3]()

                # carry for chunk 0 = QKV(1); wproj lands after x1
                qk_n, v_n = make_qkv(1)
                nc.scalar.dma_start(wproj_sb[:], wproj_r[:])
                carry = qk_n + v_n

                deferred_v = None
                for a in range(NCH):
                    # issue x(a+2)'s DMA before emitting chunk a so the
                    # transfer overlaps chunk a's execution
                    nxt = []
                    if a + 2 <= NCH - 1:
                        qk_n, v_n = make_qkv(a + 2)
                        if a + 2 == NCH - 1:
                            # defer v(3) into chunk 3 as extra PE filler
                            nxt += qk_n
                            deferred_v = v_n
                        else:
                            nxt += qk_n + v_n
                    emit_chunk(a, carry)
                    if a + 2 > NCH - 1 and deferred_v is not None:
                        nxt = deferred_v + nxt
                        deferred_v = None
                    carry = nxt + proj_steps(a, tail=(a == NCH - 1))

                # tail: keep the PE clock-gate warm through the final
                # epilogue's DVE chain, then run the last projections
                junk_mms(24, "tailwarm")
                for st in carry:
                    st()

    nc.compile()
    return nc


def _in_maps(x, w_attn, b_attn, w_proj):
    """Build the 8 per-core input maps (cid = 4*b + g)."""
    import ml_dtypes

    bf16 = ml_dtypes.bfloat16
    f = np.arange(4 * TCH) % TCH
    r = np.arange(4 * TCH) // TCH
    p = np.arange(P)
    masks = (p[:, None] <= (f - P * r)[None, :]).astype(bf16)

    wq, wk, wvv = w_attn[:, 0:C], w_attn[:, C : 2 * C], w_attn[:, 2 * C : 3 * C]
    bq = b_attn[0:C]
    if np.any(bq):
        raise NotImplementedError("nonzero q-bias not supported")

    maps = []
    for b in range(B):
        xTb = np.ascontiguousarray(x[b].T.astype(bf16))
        for g in range(4):
            s = slice(g * DC, (g + 1) * DC)
            wqk_c = np.ascontiguousarray(
                np.concatenate([0.125 * wq[:, s], wk[:, s]], axis=1).astype(bf16)
            )
            maps.append(
                {
                    "xT": xTb,
                    "wqk": wqk_c,
                    "wv": np.ascontiguousarray(wvv[:, s].astype(bf16)),
                    "wproj": np.ascontiguousarray(w_proj[s, :].astype(bf16)),
                    "masks": masks,
                }
            )
    return maps


def run(x, w_attn, b_attn, w_proj, b_proj, trace=False):
    from concourse.bass_utils import run_bass_kernel_spmd

    if "nc" not in _CACHE:
        _CACHE["nc"] = _build()
    nc = _CACHE["nc"]
    x = np.asarray(x)
    w_attn = np.asarray(w_attn)
    b_attn = np.asarray(b_attn, dtype=np.float32)
    w_proj = np.asarray(w_proj)
    maps = _in_maps(x, w_attn, b_attn, w_proj)
    r = run_bass_kernel_spmd(nc, maps, core_ids=list(range(8)), trace=trace)
    partials = [r.results[i]["out"] for i in range(8)]
    # v-bias folds exactly through softmax + projection: y = sum_p (v+bv)
    # = sum_p v + bv, so the host bias is b_proj + bv @ w_proj.
    bv = b_attn[2 * C : 3 * C]
    bp = np.asarray(b_proj, dtype=np.float32) + bv @ np.asarray(
        w_proj, dtype=np.float32
    )
    y = np.stack(
        [sum(partials[4 * b : 4 * b + 4]) + bp for b in range(B)], axis=0
    ).astype(np.float32)
    return y, r


def kernel(x, w_attn, b_attn, w_proj, b_proj):
    y, _ = run(x, w_attn, b_attn, w_proj, b_proj, trace=False)
    return y


# revision 14
# speedup vs baseline: 1.0249x; 1.0249x over previous
"""Causal self-attention (B=2, T=2048, C=1024, H=16) on 8 trn2 NeuronCores.

Sharding: Megatron tensor-parallel x data-parallel. Core cid = 4*b + g
handles batch b (of 2) and head group g (4 heads of 16): its heads' QKV
projection, attention, and the partial output projection (w_proj rows for
those heads). Host sums the 4 partials per batch and adds the bias.

Layout: everything stays transposed (host passes x[b].T) so no on-device
transposes are needed:
  - q^T/k^T come from  lhsT=w_qk[c,j],  rhs=xT[c,t]   (head pair per 128)
  - V      comes from  lhsT=xT[c,t],    rhs=w_v[c,j]  (+ persistent ones col)
  - S^T    comes from  lhsT=k^T[d,tk],  rhs=q^T[d,tq] at K=64. The two heads
           of a pair live at rows 0:64 / 64:128 of one 128-partition tile, so
           their S matmuls auto-derive tile_position (0,0)/(64,0) — emitted
           back-to-back they run CONCURRENTLY on PE row-tiles T0/T8 (2x),
           writing the two banks of one [P,2,TCH] PSUM tile.
  - y^T    comes from  lhsT=V[tk,d|1],  rhs=P^T[tk,tq] (ones col -> l)
  - out    comes from  lhsT=y^T[d,t],   rhs=w_proj[d,c]

Scale/bias folding (host): wq pre-scaled by 1/sqrt(hd); k-bias is
softmax-invariant when q-bias is 0 (dropped); v-bias folds exactly into
the host-side output bias (softmax weights sum to 1). Nonzero q-bias is
rejected.

Softmax skips max-subtraction (logits ~N(0,1), exp safe). exp runs on ACT
reading both PSUM banks of a pair-unit in one instruction (1024 cols).
Causal masking multiplies the diagonal staircase blocks (bf16, all-SBUF)
after exp; fully-masked column prefixes are skipped in S/exp/AV. The
denominator l is harvested from the V ones column; the epilogue stages
py->SBUF in one fp32 copy per head (frees the PSUM bank fast), then
reciprocal_approx_fast -> bf16 cast (DVE) -> 64-channel partition
broadcast (GpSimd) -> fused normalize-multiply into yT (DVE). No slow
GpSimd casts on the critical path.

Scheduling: one continuous PE stream. Pair-units are software-pipelined
with lookahead 2 (S pair -> exp pair -> [mask] -> AV pair two units
later); QKV of chunk a+1 and projection of chunk a-1 are paced into the
stream as filler. ~8 junk matmuls on a zeroed tile warm the PE HAM clock
gate during the initial DMA wait. Startup DMAs are spread over 4 queues
(sync/scalar/vector/gpsimd) in kt-order so the first projection matmuls
start as early as possible; x-chunk prefetch DMAs are issued one chunk
ahead on the gpsimd queue. Projection PSUM evacuations run on DVE during
steady state (ACT is exp-bound) and alternate ACT/DVE in the tail.
"""

import numpy as np

B, T, C, H = 2, 2048, 1024, 16
HD = C // H  # 64
P = 128
NKT = C // P  # 8 k-tiles over the embedding dim
TCH = 512  # t-chunk (q) width
NCH = T // TCH  # 4 q-chunks
NTB = T // P  # 16 t-blocks (k) per sequence
HPC = 4  # heads per core
DC = HPC * HD  # 256 head dims per core
NWARM = 5  # HAM warmup junk matmuls at startup

_CACHE = {}


def _build():
    import concourse.mybir as mybir
    from concourse import bacc
    from concourse.tile import TileContext

    F32 = mybir.dt.float32
    BF16 = mybir.dt.bfloat16
    AF = mybir.ActivationFunctionType

    nc = bacc.Bacc("TRN2", target_bir_lowering=False, debug=False)

    xT = nc.dram_tensor("xT", (C, T), BF16, kind="ExternalInput")
    wqk = nc.dram_tensor("wqk", (C, 4 * P), BF16, kind="ExternalInput")
    wv = nc.dram_tensor("wv", (C, DC), BF16, kind="ExternalInput")
    wproj = nc.dram_tensor("wproj", (DC, C), BF16, kind="ExternalInput")
    masks = nc.dram_tensor("masks", (P, 4 * TCH), BF16, kind="ExternalInput")
    out = nc.dram_tensor("out", (T, C), F32, kind="ExternalOutput")

    xT_r = xT[:].rearrange("(kt p) t -> p kt t", p=P)
    wqk_r = wqk[:].rearrange("(kt p) j -> p kt j", p=P)
    wv_r = wv[:].rearrange("(kt p) j -> p kt j", p=P)
    wproj_r = wproj[:].rearrange("(kt p) n -> p kt n", p=P)

    with TileContext(nc) as tc:
        with (
            tc.tile_pool(name="persist", bufs=1) as pp,
        ):
            wqk_sb = pp.tile([P, NKT, 4 * P], BF16)
            wv_sb = pp.tile([P, NKT, DC], BF16)
            wproj_sb = pp.tile([P, DC // P, C], BF16)
            masks_sb = pp.tile([P, 4 * TCH], BF16)
            # q^T/k^T head-pair tiles: pair hp rows 0:64=head 2hp, 64:128=2hp+1
            qt2 = [pp.tile([P, T], BF16, name=f"qt{i}") for i in range(2)]
            kt2 = [pp.tile([P, T], BF16, name=f"kt{i}") for i in range(2)]
            v_sb = pp.tile([P, NTB, HPC, HD + 1], BF16)  # ones col at HD
            yT_sb = pp.tile([P, DC // P, T], BF16)
            wzero = pp.tile([P, P + TCH], BF16)  # HAM warmup junk operands

            nc.vector.memset(wzero[:], 0.0)
            nc.vector.memset(v_sb[:, :, :, HD : HD + 1], 1.0)

            with (
                tc.tile_pool(name="xin", bufs=2) as xpool,
                tc.tile_pool(name="ps_s", bufs=2, space="PSUM") as ps_pool,
                tc.tile_pool(name="ps_y", bufs=2, space="PSUM") as py_pool,
                tc.tile_pool(name="ps_o", bufs=2, space="PSUM") as po_pool,
                tc.tile_pool(name="pt", bufs=4) as pt_pool,
                tc.tile_pool(name="outs", bufs=3) as o_pool,
                tc.tile_pool(name="stg", bufs=2) as stg_pool,
                tc.tile_pool(name="rec", bufs=4) as rec_pool,
                tc.tile_pool(name="rbf", bufs=2) as rbf_pool,
                tc.tile_pool(name="rb", bufs=2) as rb_pool,
            ):
                # ---------- QKV building blocks ----------
                def qk1(x_t, a2, jt):
                    """Project q (jt 0,1) or k (jt 2,3) for head pair jt%2."""
                    ch2 = slice(a2 * TCH, (a2 + 1) * TCH)
                    pq = po_pool.tile([P, TCH], F32, tag="po", name=f"pq{a2}_{jt}")
                    for kt in range(NKT):
                        nc.tensor.matmul(
                            pq[:],
                            wqk_sb[:, kt, jt * P : (jt + 1) * P],
                            x_t[:, kt, :],
                            start=(kt == 0),
                            stop=(kt == NKT - 1),
                        )
                    dst = qt2[jt] if jt < 2 else kt2[jt - 2]
                    nc.scalar.activation(dst[:, ch2], pq[:], AF.Copy)

                def v1(x_t, a2, tb):
                    pv = po_pool.tile(
                        [P, HPC, HD], F32, tag="po", name=f"pv{a2}_{tb}"
                    )
                    tg = a2 * (TCH // P) + tb
                    for kt in range(NKT):
                        nc.tensor.matmul(
                            pv[:],
                            x_t[:, kt, tb * P : (tb + 1) * P],
                            wv_sb[:, kt, :],
                            start=(kt == 0),
                            stop=(kt == NKT - 1),
                        )
                    nc.vector.tensor_copy(v_sb[:, tg, :, 0:HD], pv[:])

                def make_qkv(a2):
                    """Emit x DMA now (gpsimd queue); return qkv step list."""
                    ch2 = slice(a2 * TCH, (a2 + 1) * TCH)
                    x_t = xpool.tile(
                        [P, NKT, TCH], BF16, tag="x_tile", name=f"x{a2}"
                    )
                    nc.gpsimd.dma_start(x_t[:, 0:4, :], xT_r[:, 0:4, ch2])
                    nc.gpsimd.dma_start(x_t[:, 4:NKT, :], xT_r[:, 4:NKT, ch2])
                    qk = [
                        (lambda jt=jt: qk1(x_t, a2, jt)) for jt in (0, 2, 1, 3)
                    ]
                    v = [(lambda tb=tb: v1(x_t, a2, tb)) for tb in range(4)]
                    return qk, v

                # ---------- projection steps ----------
                def proj_steps(a, tail=False):
                    steps = []

                    def pstep(tb, ncx, cp_eng):
                        tg = a * (TCH // P) + tb
                        po = po_pool.tile(
                            [P, TCH], F32, tag="po", name=f"po{a}_{tb}_{ncx}"
                        )
                        for kt in range(DC // P):
                            nc.tensor.matmul(
                                po[:],
                                yT_sb[:, kt, tg * P : (tg + 1) * P],
                                wproj_sb[:, kt, ncx * TCH : (ncx + 1) * TCH],
                                start=(kt == 0),
                                stop=(kt == DC // P - 1),
                            )
                        ot = o_pool.tile([P, TCH], F32, tag="osb")
                        if cp_eng == "act":
                            nc.scalar.activation(ot[:], po[:], AF.Copy)
                        else:
                            nc.vector.tensor_copy(ot[:], po[:])
                        nc.sync.dma_start(
                            out[tg * P : (tg + 1) * P, ncx * TCH : (ncx + 1) * TCH],
                            ot[:],
                        )

                    for i, (tb, ncx) in enumerate(
                        (tb, ncx) for tb in range(TCH // P) for ncx in range(2)
                    ):
                        eng = ("act" if i % 2 == 0 else "dve") if tail else "dve"
                        steps.append(
                            lambda tb=tb, ncx=ncx, eng=eng: pstep(tb, ncx, eng)
                        )
                    return steps

                # ---------- attention chunk ----------
                def emit_chunk(a, carry):
                    ch = slice(a * TCH, (a + 1) * TCH)
                    nblk = 4 * a + 4
                    LOOK = 2

                    state = {}  # (hp,j) -> (pt, c0)
                    pys = {}  # (hp,s) -> py

                    def emit_S(hp, j):
                        r = j - 4 * a
                        c0 = P * r if r >= 0 else 0
                        ps = ps_pool.tile([P, 2, TCH], F32, tag="ps")
                        pt = pt_pool.tile([P, 2, TCH], BF16, tag="pt")
                        # the two 64-row matmuls run concurrently on PE row
                        # tiles (0,0)/(64,0), one PSUM bank each
                        for s in range(2):
                            hr = 64 * s
                            nc.tensor.matmul(
                                ps[:, s, c0:],
                                kt2[hp][hr : hr + 64, j * P : (j + 1) * P],
                                qt2[hp][hr : hr + 64, a * TCH + c0 : (a + 1) * TCH],
                                start=True,
                                stop=True,
                            )
                        if r >= 0:
                            nc.scalar.activation(
                                pt[:, :, c0:], ps[:, :, c0:], AF.Exp
                            )
                            for s in range(2):
                                nc.vector.tensor_mul(
                                    pt[:, s, c0:],
                                    pt[:, s, c0:],
                                    masks_sb[:, r * TCH + c0 : (r + 1) * TCH],
                                )
                        else:
                            nc.scalar.activation(pt[:], ps[:], AF.Exp)
                        state[(hp, j)] = (pt, c0)

                    def emit_AV(hp, j):
                        pt, c0 = state.pop((hp, j))
                        for s in range(2):
                            if j == 0:
                                pys[(hp, s)] = py_pool.tile(
                                    [P, TCH], F32, tag="py", name=f"py{a}_{hp}_{s}"
                                )
                            nc.tensor.matmul(
                                pys[(hp, s)][0 : HD + 1, c0:],
                                v_sb[:, j, 2 * hp + s, :],
                                pt[:, s, c0:],
                                start=(j == 0),
                                stop=(j == nblk - 1),
                            )

                    def epilogue(hp):
                        # stage py -> SBUF (bf16 y rows + fp32 l row frees the
                        # PSUM bank fast), then recip/cast/bcast/normalize.
                        # The very last epilogue has nothing left to overlap
                        # with, so offload part of its chain to the idle ACT.
                        last = a == NCH - 1 and hp == 1
                        stgs = []
                        for s in range(2):
                            py = pys.pop((hp, s))
                            stg = stg_pool.tile([HD, TCH], BF16, tag="stg")
                            if last:
                                nc.scalar.activation(stg[:], py[0:HD, :], AF.Copy)
                            else:
                                nc.vector.tensor_copy(stg[:], py[0:HD, :])
                            lrow = rec_pool.tile([1, TCH], F32, tag="lrow")
                            nc.vector.tensor_copy(lrow[:], py[HD : HD + 1, :])
                            stgs.append((stg, lrow))
                        for s in range(2):
                            stg, lrow = stgs[s]
                            hr = 64 * s
                            rec = rec_pool.tile([1, TCH], F32, tag="rec")
                            with nc.allow_low_precision(
                                reason="approx recip of softmax denominator"
                            ):
                                nc.vector.reciprocal_approx_fast(rec[:], lrow[:])
                            rbf = rbf_pool.tile([1, TCH], BF16, tag="rbf")
                            nc.vector.tensor_copy(rbf[:], rec[:])
                            rb = rb_pool.tile([64, TCH], BF16, tag="rb")
                            nc.gpsimd.partition_broadcast(rb[:], rbf[:])
                            nc.vector.tensor_mul(
                                yT_sb[hr : hr + 64, hp, ch], stg[:], rb[:]
                            )

                    units = [(hp, j) for hp in range(2) for j in range(nblk)]
                    slots = len(units) + LOOK
                    nf = len(carry)
                    popped = 0
                    for i in range(slots):
                        if i < len(units):
                            emit_S(*units[i])
                        if i >= LOOK:
                            hp, j = units[i - LOOK]
                            emit_AV(hp, j)
                            if j == nblk - 1:
                                epilogue(hp)
                        tgt = min(nf, (i + 1) * nf // slots + 1)
                        while popped < tgt:
                            carry[popped]()
                            popped += 1
                    while popped < nf:
                        carry[popped]()
                        popped += 1

                # ---------- prologue: startup DMAs on 3 queues ----------
                # fine-grained per-2-kt transfers: the Tile dependency is
                # per-DMA, so small pieces let the first matmuls start as
                # soon as their slice lands instead of waiting for 512KB
                x0 = xpool.tile([P, NKT, TCH], BF16, tag="x_tile", name="x0")
                for kt in range(0, 6, 2):
                    nc.sync.dma_start(
                        x0[:, kt : kt + 2, :], xT_r[:, kt : kt + 2, 0:TCH]
                    )
                    nc.scalar.dma_start(
                        wqk_sb[:, kt : kt + 2, :], wqk_r[:, kt : kt + 2, :]
                    )
                nc.gpsimd.dma_start(x0[:, 6:NKT, :], xT_r[:, 6:NKT, 0:TCH])
                nc.gpsimd.dma_start(wqk_sb[:, 6:NKT, :], wqk_r[:, 6:NKT, :])
                nc.gpsimd.dma_start(wv_sb[:, 0:4, :], wv_r[:, 0:4, :])
                nc.sync.dma_start(wv_sb[:, 4:NKT, :], wv_r[:, 4:NKT, :])
                nc.gpsimd.dma_start(masks_sb[:], masks[:])

                # HAM warmup: junk matmuls on the zeroed tile keep the PE
                # busy (and the clock gate warm) while DMAs or the final
                # epilogue are in flight
                def junk_mms(n, label):
                    for w in range(n):
                        pw = po_pool.tile(
                            [P, TCH], F32, tag="po", name=f"{label}{w}"
                        )
                        nc.tensor.matmul(
                            pw[:],
                            wzero[:, 0:P],
                            wzero[:, P : P + TCH],
                            start=True,
                            stop=True,
                        )

                junk_mms(NWARM, "warm")

                # QKV(0) inline: heads 0,1 q/k first, then V, then heads 2,3
                qk0, v0 = [
                    (lambda jt=jt: qk1(x0, 0, jt)) for jt in (0, 2, 1, 3)
                ], [(lambda tb=tb: v1(x0, 0, tb)) for tb in range(4)]
                qk0[0]()
                qk0[1]()
                for st in v0:
                    st()
                qk0[2]()
                qk0[3]()

                # carry for chunk 0 = QKV(1); wproj lands after x1
                qk_n, v_n = make_qkv(1)
                nc.scalar.dma_start(wproj_sb[:], wproj_r[:])
                carry = qk_n + v_n

                deferred_v = None
                for a in range(NCH):
                    # issue x(a+2)'s DMA before emitting chunk a so the
                    # transfer overlaps chunk a's execution
                    nxt = []
                    if a + 2 <= NCH - 1:
                        qk_n, v_n = make_qkv(a + 2)
                        if a + 2 == NCH - 1:
                            # defer v(3) into chunk 3 as extra PE filler
                            nxt += qk_n
                            deferred_v = v_n
                        else:
                            nxt += qk_n + v_n
                    emit_chunk(a, carry)
                    if a + 2 > NCH - 1 and deferred_v is not None:
                        nxt = deferred_v + nxt
                        deferred_v = None
                    carry = nxt + proj_steps(a, tail=(a == NCH - 1))

                # tail: keep the PE clock-gate warm through the final
                # epilogue's DVE chain, then run the last projections
                junk_mms(24, "tailwarm")
                for st in carry:
                    st()

    nc.compile()
    return nc


def _in_maps(x, w_attn, b_attn, w_proj):
    """Build the 8 per-core input maps (cid = 4*b + g)."""
    import ml_dtypes

    bf16 = ml_dtypes.bfloat16
    f = np.arange(4 * TCH) % TCH
    r = np.arange(4 * TCH) // TCH
    p = np.arange(P)
    masks = (p[:, None] <= (f - P * r)[None, :]).astype(bf16)

    wq, wk, wvv = w_attn[:, 0:C], w_attn[:, C : 2 * C], w_attn[:, 2 * C : 3 * C]
    bq = b_attn[0:C]
    if np.any(bq):
        raise NotImplementedError("nonzero q-bias not supported")

    maps = []
    for b in range(B):
        xTb = np.ascontiguousarray(x[b].T.astype(bf16))
        for g in range(4):
            s = slice(g * DC, (g + 1) * DC)
            wqk_c = np.ascontiguousarray(
                np.concatenate([0.125 * wq[:, s], wk[:, s]], axis=1).astype(bf16)
            )
            maps.append(
                {
                    "xT": xTb,
                    "wqk": wqk_c,
                    "wv": np.ascontiguousarray(wvv[:, s].astype(bf16)),
                    "wproj": np.ascontiguousarray(w_proj[s, :].astype(bf16)),
                    "masks": masks,
                }
            )
    return maps


def run(x, w_attn, b_attn, w_proj, b_proj, trace=False):
    from concourse.bass_utils import run_bass_kernel_spmd

    if "nc" not in _CACHE:
        _CACHE["nc"] = _build()
    nc = _CACHE["nc"]
    x = np.asarray(x)
    w_attn = np.asarray(w_attn)
    b_attn = np.asarray(b_attn, dtype=np.float32)
    w_proj = np.asarray(w_proj)
    maps = _in_maps(x, w_attn, b_attn, w_proj)
    r = run_bass_kernel_spmd(nc, maps, core_ids=list(range(8)), trace=trace)
    partials = [r.results[i]["out"] for i in range(8)]
    # v-bias folds exactly through softmax + projection: y = sum_p (v+bv)
    # = sum_p v + bv, so the host bias is b_proj + bv @ w_proj.
    bv = b_attn[2 * C : 3 * C]
    bp = np.asarray(b_proj, dtype=np.float32) + bv @ np.asarray(
        w_proj, dtype=np.float32
    )
    y = np.stack(
        [sum(partials[4 * b : 4 * b + 4]) + bp for b in range(B)], axis=0
    ).astype(np.float32)
    return y, r


def kernel(x, w_attn, b_attn, w_proj, b_proj):
    y, _ = run(x, w_attn, b_attn, w_proj, b_proj, trace=False)
    return y
